# revision 57
# baseline (speedup 1.0000x reference)
"""BiAttention (BiDAF-style) kernel for Trainium2, 8 NeuronCores.

Reference math (T=4096, d=512):
    context  = x[0,0]; question = x[1,0]
    S[i,j]   = w1.c_i + w2.q_j + (c_i*w3).q_j
    A        = softmax_j(S)          # w1.c_i is constant per row -> cancels
    U_A      = A @ question
    b        = max_j A[i,j]
    h        = b @ context           # global over T -> one AllReduce
    G        = [context, U_A, context*U_A, context*h]

Sharding: context rows (rows of S/A/U_A/G) split across 8 cores (512 each);
question replicated; h AllReduced (2 KB).

Numerics: the S matmul runs as THREE fp8e4m3 DoubleRow streams
(W8@q8 + W8@r8 + V8@q8, where r8 is the fp8 residual of q and V8 the fp8
residual of W = 64*(c*w3 + w2)), recovering ~fp12 effective precision at
fp8 DoubleRow speed (0.5 cyc/row).  E=exp(S/64) is stored fp8 in a
pair-permuted layout; U_A = E@q8 runs fp8 DoubleRowSwInterleave with E.T
produced by fp16-punned PE transposes through an anti-diagonal
permutation (pre-reversing the columns that SwInterleave re-reverses).
b uses the f32 row-max of S taken straight off the psum.
End-to-end rel err ~2.2e-3 (tolerance 2e-2).

All input-side operand layouts (fp8 casts, residuals, pun-transposed q,
W8/V8) are prepared host-side in kernel() and DMA'd in, so the chip
spends no time marshaling inputs.  Phase-2 work (E.T, U_A, G) for
i-block k is interleaved between the S-matmul groups of i-block k+1 to
keep every engine busy.
"""

import numpy as np
import ml_dtypes

import concourse.bass as bass
import concourse.mybir as mybir
import concourse.tile as tile
from concourse import bacc
from concourse.bass_utils import run_bass_kernel_spmd
from concourse.masks import make_identity

F32 = mybir.dt.float32
F16 = mybir.dt.float16
F8 = mybir.dt.float8e4
U16 = mybir.dt.uint16
BF16 = mybir.dt.bfloat16
AF = mybir.ActivationFunctionType
ALU = mybir.AluOpType
DR = mybir.MatmulPerfMode.DoubleRow
DRS = mybir.MatmulPerfMode.DoubleRowSwInterleave

F8NP = ml_dtypes.float8_e4m3

T = 4096
D = 512
NCORES = 8
TL = T // NCORES          # 512 local context rows per core
P = 128
NIB = TL // P             # 4 i-blocks of 128 rows
NJT = T // P              # 32 j-tiles of 128
SC = 64.0                 # W scale; exp() divides it back out

NGRP = 4                  # psum groups per i-block ([128,1024] = 2 js each)
TPG = NJT // NGRP         # 8 j-tiles per group


def build_kernel(collective=True, compile=True):
    nc = bacc.Bacc("TRN2", target_bir_lowering=False, debug=False,
                   num_devices=NCORES if collective else 1)

    qnat_d = nc.dram_tensor("qnat", [P, NJT, D], F8, kind="ExternalInput").ap()
    qt_d = nc.dram_tensor("qt", [P, 2, 2, T], F8, kind="ExternalInput").ap()
    rt_d = nc.dram_tensor("rt", [P, 2, 2, T], F8, kind="ExternalInput").ap()
    c16_d = nc.dram_tensor("c16", [P, NIB, D], F16, kind="ExternalInput").ap()
    w8_d = nc.dram_tensor("w8", [P, 2 * NIB, 2, P], F8, kind="ExternalInput").ap()
    v8_d = nc.dram_tensor("v8", [P, 2 * NIB, 2, P], F8, kind="ExternalInput").ap()
    c32_d = nc.dram_tensor("c32", [TL, D], F32, kind="ExternalInput").ap()
    g_d = nc.dram_tensor("g", [TL, 4 * D], F32, kind="ExternalOutput").ap()

    with tile.TileContext(nc) as tc:
        _emit(nc, tc, qnat_d, qt_d, rt_d, c16_d, w8_d, v8_d, c32_d, g_d,
              collective=collective)

    if compile:
        nc.compile()
    return nc


def _emit(nc, tc, qnat_d, qt_d, rt_d, c16_d, w8_d, v8_d, c32_d, g_d,
          collective=True):
    from contextlib import ExitStack
    ctx = ExitStack()
    consts = ctx.enter_context(tc.tile_pool(name="consts", bufs=1))
    epool = ctx.enter_context(tc.tile_pool(name="epool", bufs=1))
    stat = ctx.enter_context(tc.tile_pool(name="stat", bufs=4))
    gout = ctx.enter_context(tc.tile_pool(name="gout", bufs=1))
    spool = ctx.enter_context(tc.tile_pool(name="spool", bufs=3, space="PSUM"))
    uapool = ctx.enter_context(tc.tile_pool(name="uapool", bufs=1, space="PSUM"))
    etp = ctx.enter_context(tc.tile_pool(name="etp", bufs=1, space="PSUM"))
    dram = ctx.enter_context(tc.tile_pool(name="dram", bufs=1, space="DRAM"))

    # ---- small loads first (w8/v8 gate the S matmuls) ---------------------
    w8 = consts.tile([P, 2 * NIB, 2, P], F8)
    nc.sync.dma_start(out=w8, in_=w8_d)
    v8 = consts.tile([P, 2 * NIB, 2, P], F8)
    nc.sync.dma_start(out=v8, in_=v8_d)
    ident = consts.tile([P, P], F16)
    make_identity(nc, ident)
    # anti-diagonal permutation: rev[x, y] = (x + y == 127); transposing E
    # through it pre-reverses the columns that SwInterleave will re-reverse
    rev = consts.tile([P, P], F16)
    nc.gpsimd.memset(rev, 0.0)
    nc.gpsimd.affine_select(out=rev, in_=rev,
                            compare_op=ALU.not_equal, fill=1.0,
                            base=-(P - 1), pattern=[[1, P]],
                            channel_multiplier=1)
    # pull the exp ACT table load into the startup window
    warm = consts.tile([1, 1], F32)
    nc.vector.memset(warm, 0.0)
    nc.scalar.activation(out=warm, in_=warm, func=AF.Exp)
    # PE p-state warm-up matmuls fill the cold-start DMA wait
    wa = consts.tile([P, P], BF16)
    nc.vector.memset(wa, 0.0)
    wb = consts.tile([P, D], BF16)
    nc.vector.memset(wb, 0.0)
    for wi in range(2):
        wps = uapool.tile([P, D], F32, tag="ua", name=f"wps{wi}")
        nc.tensor.matmul(wps, lhsT=wa, rhs=wb, start=True, stop=False)
        nc.tensor.matmul(wps, lhsT=wa, rhs=wb, start=False, stop=False)
        nc.tensor.matmul(wps, lhsT=wa, rhs=wb, start=False, stop=True)

    # ---- big loads, chunked so phase 1 can start early --------------------
    # qt/rt layout [p, g, lo, j]: element = a8[j, g*256 + 2p + lo]
    qt = consts.tile([P, 2, 2, T], F8)
    rt = consts.tile([P, 2, 2, T], F8)
    qnat = consts.tile([P, NJT, D], F8)
    NCH = 8
    jch = T // NCH
    jc = NJT // NCH
    for ch in range(NCH):
        sl = slice(ch * jch, (ch + 1) * jch)
        nc.sync.dma_start(out=qt[:, :, :, sl], in_=qt_d[:, :, :, sl])
        nc.sync.dma_start(out=rt[:, :, :, sl], in_=rt_d[:, :, :, sl])
    c16 = consts.tile([P, NIB, D], F16)
    nc.sync.dma_start(out=c16, in_=c16_d)
    for ch in range(NCH):
        nc.sync.dma_start(out=qnat[:, ch * jc:(ch + 1) * jc],
                          in_=qnat_d[:, ch * jc:(ch + 1) * jc])

    # G block 0: pure DRAM->DRAM copy of context, queued behind the loads
    nc.sync.dma_start(out=g_d[:, 0:D], in_=c32_d)

    # ---- persistent E / stats tiles --------------------------------------
    e_sb = []
    zpart = []
    smax = []
    etT = []
    for ib in range(NIB):
        e_sb.append(epool.tile([P, T], F8, tag=f"e{ib}", name=f"e{ib}"))
        zpart.append(stat.tile([P, NGRP], F32, tag=f"zp{ib}", name=f"zp{ib}"))
        smax.append(stat.tile([P, NGRP], F32, tag=f"sm{ib}", name=f"sm{ib}"))
        etT.append(epool.tile([P, NJT // 2, P, 2], F8, tag=f"et{ib}",
                              name=f"et{ib}"))

    zinvs = []
    b16s = []
    ua_pss = {}

    g_pack = gout.tile([P, NIB, 3 * D], F16)
    h_parts = stat.tile([P, NIB, NIB], F32, tag="hparts", name="h_parts")

    def emit_s_group(ib, grp):
        ps = spool.tile([P, 2 * D], F32, tag="s")
        streams = [(w8, qt, 0), (w8, rt, 0), (v8, qt, 0),
                   (w8, qt, 1), (w8, rt, 1), (v8, qt, 1)]
        for half in range(2):
            js = grp * 2 + half
            col = half * D
            for si, (lhs, rhsrc, g) in enumerate(streams):
                nc.tensor.matmul(
                    ps[:, col:col + D],
                    lhsT=lhs[:, ib * 2 + g],
                    rhs=rhsrc[:, g, :, js * D:(js + 1) * D],
                    start=(si == 0),
                    stop=(si == len(streams) - 1),
                    perf_mode=DR,
                    skip_group_check=True,
                )
        # f32 row-max of S straight off the psum (feeds b at f32 quality)
        nc.vector.tensor_reduce(
            out=smax[ib][:, grp:grp + 1], in_=ps,
            axis=mybir.AxisListType.X, op=ALU.max)
        # exp with the sigma-scatter: psum col (t, f) -> e_sb offset
        # (grp*4 + t//2)*256 + 2f + (t%2)  [t-hi stride 256, t-lo 1, f 2]
        e_view = e_sb[ib][:, grp * 1024:(grp + 1) * 1024].rearrange(
            "p (th f tl) -> p th tl f", th=TPG // 2, f=P, tl=2)
        ps_view = ps.rearrange("p (th tl f) -> p th tl f",
                               th=TPG // 2, tl=2, f=P)
        nc.scalar.activation(out=e_view, in_=ps_view, func=AF.Exp,
                             scale=1.0 / SC,
                             accum_out=zpart[ib][:, grp:grp + 1])

    def emit_stats(ib):
        # Z and 1/Z; b numerator from the f32 S row-max
        z = stat.tile([P, 1], F32, tag="z")
        nc.vector.tensor_reduce(out=z, in_=zpart[ib],
                                axis=mybir.AxisListType.X, op=ALU.add)
        zinv = stat.tile([P, 1], F32, tag=f"zi{ib}", name=f"zi{ib}")
        nc.vector.reciprocal(out=zinv, in_=z)
        zinvs.append(zinv)
        sm = stat.tile([P, 1], F32, tag="sm1")
        nc.vector.tensor_reduce(out=sm, in_=smax[ib],
                                axis=mybir.AxisListType.X, op=ALU.max)
        eb = stat.tile([P, 1], F32, tag="eb")
        nc.scalar.activation(out=eb, in_=sm, func=AF.Exp, scale=1.0 / SC)
        b16 = stat.tile([P, 1], F16, tag=f"b{ib}", name=f"b{ib}")
        nc.vector.tensor_tensor(out=b16, in0=eb, in1=zinv, op=ALU.mult)
        b16s.append(b16)

    def emit_phase2_piece(ib, piece):
        """Phase-2 work for i-block ib, interleaved between S groups of
        ib+1: 0=E.T (DMA route or PE half 1), 1=PE half 2, 2=UA K0..7,
        3=UA K8..15 + b/h partial + ua16/cu + G write."""
        e_u16 = e_sb[ib][:, 0:T].bitcast(F16)
        et_u16 = etT[ib].rearrange("p a b c -> p (a b c)").bitcast(F16)
        if piece in (0, 1):
            eps = etp.tile([P, 1024], F16, tag="t")
            for tt in range(8):
                idx = piece * 8 + tt
                nc.tensor.transpose(eps[:, tt * P:(tt + 1) * P],
                                    e_u16[:, idx * P:(idx + 1) * P], rev)
            nc.vector.tensor_copy(
                out=et_u16[:, piece * 1024:(piece + 1) * 1024], in_=eps)
            return
        if piece == 2:
            ua_ps = uapool.tile([P, D], F32, tag="ua", name=f"ua{ib}")
            ua_pss[ib] = ua_ps
            for K in range(8):
                nc.tensor.matmul(
                    ua_ps,
                    lhsT=etT[ib][:, K],
                    rhs=qnat[:, 2 * K:2 * K + 2, :],
                    start=(K == 0), stop=False,
                    perf_mode=DRS,
                )
            return
        # piece 3
        # h partial first: 4 tiny matmuls into the etp ring, then to SBUF
        hp_ps = etp.tile([P, NIB], F32, tag="t", name=f"hp{ib}")
        for dc in range(NIB):
            nc.tensor.matmul(hp_ps[:, dc:dc + 1],
                             lhsT=c16[:, ib, dc * P:(dc + 1) * P],
                             rhs=b16s[ib],
                             start=(dc == 0), stop=(dc == NIB - 1),
                             skip_group_check=True)
        nc.scalar.activation(out=h_parts[:, :, ib], in_=hp_ps, func=AF.Copy)
        ua_ps = ua_pss[ib]
        for K in range(8, NJT // 2):
            nc.tensor.matmul(
                ua_ps,
                lhsT=etT[ib][:, K],
                rhs=qnat[:, 2 * K:2 * K + 2, :],
                start=False, stop=(K == NJT // 2 - 1),
                perf_mode=DRS,
            )
        # ua16 = ua * zinv (DVE) ; cu = c16*ua16 (Pool) ; ship both
        nc.vector.tensor_scalar(g_pack[:, ib, 0:D], ua_ps, zinvs[ib],
                                None, ALU.mult)
        nc.gpsimd.tensor_tensor(out=g_pack[:, ib, D:2 * D],
                                in0=c16[:, ib], in1=g_pack[:, ib, 0:D],
                                op=ALU.mult)
        nc.gpsimd.dma_start(
            out=g_d[ib * P:(ib + 1) * P, D:3 * D],
            in_=g_pack[:, ib, 0:2 * D])

    # ---- main pipeline (phase-2 of ib rides under S of ib+1 / ib+2) ------
    for grp in range(NGRP):
        emit_s_group(0, grp)
    for ib in range(NIB):
        for grp in range(NGRP):
            if ib + 1 < NIB:
                emit_s_group(ib + 1, grp)
            if grp == 2:
                emit_phase2_piece(ib, 0)
            elif grp == 3:
                emit_phase2_piece(ib, 1)
                emit_stats(ib)
            elif ib >= 1 and grp == 0:
                emit_phase2_piece(ib - 1, 2)
            elif ib >= 1 and grp == 1:
                emit_phase2_piece(ib - 1, 3)
    emit_phase2_piece(NIB - 1, 2)
    emit_phase2_piece(NIB - 1, 3)

    # ---- h AllReduce, then G block 3 -------------------------------------
    h_sb = stat.tile([P, NIB], F32, tag="h_sb")
    nc.vector.tensor_reduce(out=h_sb, in_=h_parts,
                            axis=mybir.AxisListType.X, op=ALU.add)
    hp_dram = dram.tile([D], F32)
    hs_dram = dram.tile([D], F32)
    hp_ap = hp_dram[:]
    nc.sync.dma_start(out=hp_ap.rearrange("(dc p) -> p dc", p=P), in_=h_sb)
    if collective:
        nc.gpsimd.collective_compute(
            "AllReduce", ALU.add,
            replica_groups=[list(range(NCORES))],
            ins=[hp_dram.opt()], outs=[hs_dram.opt()],
        )
    else:
        nc.sync.dma_start(out=hs_dram[:], in_=hp_dram[:])
    hs_ap = hs_dram[:]
    hb = consts.tile([P, D], F32)
    nc.sync.dma_start(
        out=hb,
        in_=bass.AP(tensor=hs_ap.tensor, offset=hs_ap.offset,
                    ap=[[0, P], [1, D]]),
    )
    for ib in range(NIB):
        chx = gout.tile([P, D], F32, tag=f"ch{ib}", name=f"ch{ib}")
        nc.vector.tensor_tensor(out=chx, in0=c16[:, ib], in1=hb, op=ALU.mult)
        nc.sync.dma_start(out=g_d[ib * P:(ib + 1) * P, 3 * D:4 * D], in_=chx)

    ctx.close()


_NC_CACHE = {}


def _get_nc():
    if "nc" not in _NC_CACHE:
        _NC_CACHE["nc"] = build_kernel()
    return _NC_CACHE["nc"]


def _host_prep(x: np.ndarray, kern: np.ndarray):
    context = np.ascontiguousarray(x[0, 0]).astype(np.float32)   # (T, D)
    question = np.ascontiguousarray(x[1, 0]).astype(np.float32)  # (T, D)
    w = np.asarray(kern, dtype=np.float32)
    w2 = w[D:2 * D] * SC
    w3 = w[2 * D:3 * D] * SC

    q8 = question.astype(F8NP)
    r8 = (question - q8.astype(np.float32)).astype(F8NP)

    def punT(a8):
        # [T, D] fp8 -> [p, g, lo, j]: val = a8[j, g*256 + 2p + lo]
        v = a8.reshape(T, 2, P, 2)               # j, g, p, lo
        return np.ascontiguousarray(v.transpose(2, 1, 3, 0))

    def punW(a8):
        # [TL, D] fp8 -> [p, K=(ib,g), lo, f]: val = a8[ib*128+f, g*256+2p+lo]
        v = a8.reshape(NIB, P, 2, P, 2)          # ib, f, g, p, lo
        return np.ascontiguousarray(v.transpose(3, 0, 2, 4, 1)
                                    .reshape(P, 2 * NIB, 2, P))

    qnat = np.ascontiguousarray(
        q8.reshape(NJT, P, D).transpose(1, 0, 2))          # [p, jt, d]
    qt = punT(q8)
    rt = punT(r8)

    in_maps = []
    for core in range(NCORES):
        c = np.ascontiguousarray(context[core * TL:(core + 1) * TL])
        c16f = c.astype(np.float16)
        c16 = np.ascontiguousarray(
            c16f.reshape(NIB, P, D).transpose(1, 0, 2))    # [p, ib, d]
        wfull = (c16f.astype(np.float32) * w3[None, :] + w2[None, :])
        w8 = wfull.astype(F8NP)
        v8 = (wfull - w8.astype(np.float32)).astype(F8NP)
        in_maps.append({
            "qnat": qnat, "qt": qt, "rt": rt,
            "c16": c16, "w8": punW(w8), "v8": punW(v8), "c32": c,
        })
    return in_maps


def kernel(x: np.ndarray, kernel: np.ndarray) -> np.ndarray:
    nc = _get_nc()
    in_maps = _host_prep(x, kernel)
    res = run_bass_kernel_spmd(nc, in_maps, core_ids=list(range(NCORES)))
    g = np.concatenate([res.results[core]["g"] for core in range(NCORES)],
                       axis=0)
    return g.astype(np.float32)


# revision 58
# speedup vs baseline: 1.0435x; 1.0435x over previous
"""BiAttention (BiDAF-style) kernel for Trainium2, 8 NeuronCores.

Reference math (T=4096, d=512):
    context  = x[0,0]; question = x[1,0]
    S[i,j]   = w1.c_i + w2.q_j + (c_i*w3).q_j
    A        = softmax_j(S)          # w1.c_i is constant per row -> cancels
    U_A      = A @ question
    b        = max_j A[i,j]
    h        = b @ context           # global over T -> one AllReduce
    G        = [context, U_A, context*U_A, context*h]

Sharding: context rows (rows of S/A/U_A/G) split across 8 cores (512 each);
question replicated; h AllReduced (2 KB).

Numerics: the S matmul runs as THREE fp8e4m3 DoubleRow streams
(W8@q8 + W8@r8 + V8@q8, where r8 is the fp8 residual of q and V8 the fp8
residual of W = 64*(c*w3 + w2)), recovering ~fp12 effective precision at
fp8 DoubleRow speed (0.5 cyc/row).  E=exp(S/64) is stored fp8 in a
pair-permuted layout; U_A = E@q8 runs fp8 DoubleRowSwInterleave with E.T
produced by fp16-punned PE transposes through an anti-diagonal
permutation (pre-reversing the columns that SwInterleave re-reverses).
b uses the f32 row-max of S taken straight off the psum.
End-to-end rel err ~2.2e-3 (tolerance 2e-2).

All input-side operand layouts (fp8 casts, residuals, pun-transposed q,
W8/V8) are prepared host-side in kernel() and DMA'd in, so the chip
spends no time marshaling inputs.  Phase-2 work (E.T, U_A, G) for
i-block k is interleaved between the S-matmul groups of i-block k+1 to
keep every engine busy.
"""

import numpy as np
import ml_dtypes

import concourse.bass as bass
import concourse.mybir as mybir
import concourse.tile as tile
from concourse import bacc
from concourse.bass_utils import run_bass_kernel_spmd
from concourse.masks import make_identity

F32 = mybir.dt.float32
F16 = mybir.dt.float16
F8 = mybir.dt.float8e4
U16 = mybir.dt.uint16
BF16 = mybir.dt.bfloat16
AF = mybir.ActivationFunctionType
ALU = mybir.AluOpType
DR = mybir.MatmulPerfMode.DoubleRow
DRS = mybir.MatmulPerfMode.DoubleRowSwInterleave

F8NP = ml_dtypes.float8_e4m3

T = 4096
D = 512
NCORES = 8
TL = T // NCORES          # 512 local context rows per core
P = 128
NIB = TL // P             # 4 i-blocks of 128 rows
NJT = T // P              # 32 j-tiles of 128
SC = 64.0                 # W scale; exp() divides it back out

NGRP = 4                  # psum groups per i-block ([128,1024] = 2 js each)
TPG = NJT // NGRP         # 8 j-tiles per group


def build_kernel(collective=True, compile=True):
    nc = bacc.Bacc("TRN2", target_bir_lowering=False, debug=False,
                   num_devices=NCORES if collective else 1)

    qnat_d = nc.dram_tensor("qnat", [P, NJT, D], F8, kind="ExternalInput").ap()
    qt_d = nc.dram_tensor("qt", [P, 2, 2, T], F8, kind="ExternalInput").ap()
    rt_d = nc.dram_tensor("rt", [P, 2, 2, T], F8, kind="ExternalInput").ap()
    c16_d = nc.dram_tensor("c16", [P, NIB, D], F16, kind="ExternalInput").ap()
    w8_d = nc.dram_tensor("w8", [P, 2 * NIB, 2, P], F8, kind="ExternalInput").ap()
    v8_d = nc.dram_tensor("v8", [P, 2 * NIB, 2, P], F8, kind="ExternalInput").ap()
    c32_d = nc.dram_tensor("c32", [TL, D], F32, kind="ExternalInput").ap()
    g_d = nc.dram_tensor("g", [TL, 4 * D], F32, kind="ExternalOutput").ap()

    with tile.TileContext(nc) as tc:
        _emit(nc, tc, qnat_d, qt_d, rt_d, c16_d, w8_d, v8_d, c32_d, g_d,
              collective=collective)

    if compile:
        nc.compile()
    return nc


def _emit(nc, tc, qnat_d, qt_d, rt_d, c16_d, w8_d, v8_d, c32_d, g_d,
          collective=True):
    from contextlib import ExitStack
    ctx = ExitStack()
    consts = ctx.enter_context(tc.tile_pool(name="consts", bufs=1))
    epool = ctx.enter_context(tc.tile_pool(name="epool", bufs=1))
    stat = ctx.enter_context(tc.tile_pool(name="stat", bufs=4))
    gout = ctx.enter_context(tc.tile_pool(name="gout", bufs=1))
    spool = ctx.enter_context(tc.tile_pool(name="spool", bufs=3, space="PSUM"))
    uapool = ctx.enter_context(tc.tile_pool(name="uapool", bufs=1, space="PSUM"))
    etp = ctx.enter_context(tc.tile_pool(name="etp", bufs=1, space="PSUM"))
    dram = ctx.enter_context(tc.tile_pool(name="dram", bufs=1, space="DRAM"))

    # ---- small loads first (w8/v8 gate the S matmuls) ---------------------
    w8 = consts.tile([P, 2 * NIB, 2, P], F8)
    nc.sync.dma_start(out=w8, in_=w8_d)
    v8 = consts.tile([P, 2 * NIB, 2, P], F8)
    nc.sync.dma_start(out=v8, in_=v8_d)
    ident = consts.tile([P, P], F16)
    make_identity(nc, ident)
    # anti-diagonal permutation: rev[x, y] = (x + y == 127); transposing E
    # through it pre-reverses the columns that SwInterleave will re-reverse
    rev = consts.tile([P, P], F16)
    nc.gpsimd.memset(rev, 0.0)
    nc.gpsimd.affine_select(out=rev, in_=rev,
                            compare_op=ALU.not_equal, fill=1.0,
                            base=-(P - 1), pattern=[[1, P]],
                            channel_multiplier=1)
    # pull the exp ACT table load into the startup window
    warm = consts.tile([1, 1], F32)
    nc.vector.memset(warm, 0.0)
    nc.scalar.activation(out=warm, in_=warm, func=AF.Exp)
    # PE p-state warm-up matmuls fill the cold-start DMA wait
    wa = consts.tile([P, P], BF16)
    nc.vector.memset(wa, 0.0)
    wb = consts.tile([P, D], BF16)
    nc.vector.memset(wb, 0.0)
    for wi in range(2):
        wps = uapool.tile([P, D], F32, tag="ua", name=f"wps{wi}")
        nc.tensor.matmul(wps, lhsT=wa, rhs=wb, start=True, stop=False)
        nc.tensor.matmul(wps, lhsT=wa, rhs=wb, start=False, stop=False)
        nc.tensor.matmul(wps, lhsT=wa, rhs=wb, start=False, stop=True)

    # ---- big loads, chunked so phase 1 can start early --------------------
    # qt/rt layout [p, g, lo, j]: element = a8[j, g*256 + 2p + lo]
    qt = consts.tile([P, 2, 2, T], F8)
    rt = consts.tile([P, 2, 2, T], F8)
    qnat = consts.tile([P, NJT, D], F8)
    NCH = 8
    jch = T // NCH
    jc = NJT // NCH
    for ch in range(NCH):
        sl = slice(ch * jch, (ch + 1) * jch)
        nc.sync.dma_start(out=qt[:, :, :, sl], in_=qt_d[:, :, :, sl])
        nc.sync.dma_start(out=rt[:, :, :, sl], in_=rt_d[:, :, :, sl])
    c16 = consts.tile([P, NIB, D], F16)
    nc.sync.dma_start(out=c16, in_=c16_d)
    for ch in range(NCH):
        nc.sync.dma_start(out=qnat[:, ch * jc:(ch + 1) * jc],
                          in_=qnat_d[:, ch * jc:(ch + 1) * jc])

    # G block 0: pure DRAM->DRAM copy of context, queued behind the loads
    nc.sync.dma_start(out=g_d[:, 0:D], in_=c32_d)

    # ---- persistent E / stats tiles --------------------------------------
    e_sb = []
    zpart = []
    smax = []
    ebs = []
    etT = []
    for ib in range(NIB):
        e_sb.append(epool.tile([P, T], F8, tag=f"e{ib}", name=f"e{ib}"))
        zpart.append(stat.tile([P, NGRP], F32, tag=f"zp{ib}", name=f"zp{ib}"))
        smax.append(stat.tile([P, NGRP], F32, tag=f"sm{ib}", name=f"sm{ib}"))
        ebs.append(stat.tile([P, NGRP], F32, tag=f"eb{ib}", name=f"eb{ib}"))
        etT.append(epool.tile([P, NJT // 2, P, 2], F8, tag=f"et{ib}",
                              name=f"et{ib}"))

    zinvs = []
    b16s = []
    ua_pss = {}

    g_pack = gout.tile([P, NIB, 3 * D], F16)
    h_parts = stat.tile([P, NIB, NIB], F32, tag="hparts", name="h_parts")

    def emit_s_group(ib, grp):
        ps = spool.tile([P, 2 * D], F32, tag="s")
        streams = [(w8, qt, 0), (w8, rt, 0), (v8, qt, 0),
                   (w8, qt, 1), (w8, rt, 1), (v8, qt, 1)]
        for half in range(2):
            js = grp * 2 + half
            col = half * D
            for si, (lhs, rhsrc, g) in enumerate(streams):
                nc.tensor.matmul(
                    ps[:, col:col + D],
                    lhsT=lhs[:, ib * 2 + g],
                    rhs=rhsrc[:, g, :, js * D:(js + 1) * D],
                    start=(si == 0),
                    stop=(si == len(streams) - 1),
                    perf_mode=DR,
                    skip_group_check=True,
                )
        # f32 row-max of S straight off the psum (feeds b at f32 quality);
        # exp'd per group so the tail's b16 needs no ACT round-trip
        nc.vector.tensor_reduce(
            out=smax[ib][:, grp:grp + 1], in_=ps,
            axis=mybir.AxisListType.X, op=ALU.max)
        nc.scalar.activation(out=ebs[ib][:, grp:grp + 1],
                             in_=smax[ib][:, grp:grp + 1],
                             func=AF.Exp, scale=1.0 / SC)
        # exp with the sigma-scatter: psum col (t, f) -> e_sb offset
        # (grp*4 + t//2)*256 + 2f + (t%2)  [t-hi stride 256, t-lo 1, f 2]
        e_view = e_sb[ib][:, grp * 1024:(grp + 1) * 1024].rearrange(
            "p (th f tl) -> p th tl f", th=TPG // 2, f=P, tl=2)
        ps_view = ps.rearrange("p (th tl f) -> p th tl f",
                               th=TPG // 2, tl=2, f=P)
        nc.scalar.activation(out=e_view, in_=ps_view, func=AF.Exp,
                             scale=1.0 / SC,
                             accum_out=zpart[ib][:, grp:grp + 1])

    def emit_stats(ib):
        # Z and 1/Z; b numerator from the f32 S row-max
        z = stat.tile([P, 1], F32, tag="z")
        nc.vector.tensor_reduce(out=z, in_=zpart[ib],
                                axis=mybir.AxisListType.X, op=ALU.add)
        zinv = stat.tile([P, 1], F32, tag=f"zi{ib}", name=f"zi{ib}")
        nc.vector.reciprocal(out=zinv, in_=z)
        zinvs.append(zinv)
        bn = stat.tile([P, 1], F32, tag="bn")
        nc.vector.tensor_reduce(out=bn, in_=ebs[ib],
                                axis=mybir.AxisListType.X, op=ALU.max)
        b16 = stat.tile([P, 1], F16, tag=f"b{ib}", name=f"b{ib}")
        nc.vector.tensor_tensor(out=b16, in0=bn, in1=zinv, op=ALU.mult)
        b16s.append(b16)

    def emit_phase2_piece(ib, piece):
        """Phase-2 work for i-block ib, interleaved between S groups of
        ib+1: 0=E.T (DMA route or PE half 1), 1=PE half 2, 2=UA K0..7,
        3=UA K8..15 + b/h partial + ua16/cu + G write."""
        e_u16 = e_sb[ib][:, 0:T].bitcast(F16)
        et_u16 = etT[ib].rearrange("p a b c -> p (a b c)").bitcast(F16)
        if piece in (0, 1):
            eps = etp.tile([P, 1024], F16, tag="t")
            for tt in range(8):
                idx = piece * 8 + tt
                nc.tensor.transpose(eps[:, tt * P:(tt + 1) * P],
                                    e_u16[:, idx * P:(idx + 1) * P], rev)
            nc.vector.tensor_copy(
                out=et_u16[:, piece * 1024:(piece + 1) * 1024], in_=eps)
            return
        if piece == 2:
            ua_ps = uapool.tile([P, D], F32, tag="ua", name=f"ua{ib}")
            ua_pss[ib] = ua_ps
            for K in range(8):
                nc.tensor.matmul(
                    ua_ps,
                    lhsT=etT[ib][:, K],
                    rhs=qnat[:, 2 * K:2 * K + 2, :],
                    start=(K == 0), stop=False,
                    perf_mode=DRS,
                )
            return
        # piece 3
        # h partial first: 4 tiny matmuls; the last i-block uses the spool
        # ring (drained by then) to dodge the etp ring wait on the tail path
        hpool = spool if ib == NIB - 1 else etp
        htag = "s" if ib == NIB - 1 else "t"
        hp_ps = hpool.tile([P, NIB], F32, tag=htag, name=f"hp{ib}")
        for dc in range(NIB):
            nc.tensor.matmul(hp_ps[:, dc:dc + 1],
                             lhsT=c16[:, ib, dc * P:(dc + 1) * P],
                             rhs=b16s[ib],
                             start=(dc == 0), stop=(dc == NIB - 1),
                             skip_group_check=True)
        nc.scalar.activation(out=h_parts[:, :, ib], in_=hp_ps, func=AF.Copy)
        ua_ps = ua_pss[ib]
        for K in range(8, NJT // 2):
            nc.tensor.matmul(
                ua_ps,
                lhsT=etT[ib][:, K],
                rhs=qnat[:, 2 * K:2 * K + 2, :],
                start=False, stop=(K == NJT // 2 - 1),
                perf_mode=DRS,
            )
        # ua16 = ua * zinv (DVE) ; cu = c16*ua16 (Pool) ; ship both
        nc.vector.tensor_scalar(g_pack[:, ib, 0:D], ua_ps, zinvs[ib],
                                None, ALU.mult)
        nc.gpsimd.tensor_tensor(out=g_pack[:, ib, D:2 * D],
                                in0=c16[:, ib], in1=g_pack[:, ib, 0:D],
                                op=ALU.mult)
        nc.gpsimd.dma_start(
            out=g_d[ib * P:(ib + 1) * P, D:3 * D],
            in_=g_pack[:, ib, 0:2 * D])

    # ---- main pipeline (phase-2 of ib rides under S of ib+1 / ib+2) ------
    for grp in range(NGRP):
        emit_s_group(0, grp)
    for ib in range(NIB):
        for grp in range(NGRP):
            if ib + 1 < NIB:
                emit_s_group(ib + 1, grp)
            if grp == 2:
                emit_phase2_piece(ib, 0)
            elif grp == 3:
                emit_phase2_piece(ib, 1)
                emit_stats(ib)
            elif ib >= 1 and grp == 0:
                emit_phase2_piece(ib - 1, 2)
            elif ib >= 1 and grp == 1:
                emit_phase2_piece(ib - 1, 3)
    emit_phase2_piece(NIB - 1, 2)
    emit_phase2_piece(NIB - 1, 3)

    # ---- h AllReduce, then G block 3 -------------------------------------
    h_sb = stat.tile([P, NIB], F32, tag="h_sb")
    nc.vector.tensor_reduce(out=h_sb, in_=h_parts,
                            axis=mybir.AxisListType.X, op=ALU.add)
    hp_dram = dram.tile([D], F32)
    hs_dram = dram.tile([D], F32)
    hp_ap = hp_dram[:]
    nc.sync.dma_start(out=hp_ap.rearrange("(dc p) -> p dc", p=P), in_=h_sb)
    if collective:
        nc.gpsimd.collective_compute(
            "AllReduce", ALU.add,
            replica_groups=[list(range(NCORES))],
            ins=[hp_dram.opt()], outs=[hs_dram.opt()],
        )
    else:
        nc.sync.dma_start(out=hs_dram[:], in_=hp_dram[:])
    hs_ap = hs_dram[:]
    hb = consts.tile([P, D], F32)
    nc.sync.dma_start(
        out=hb,
        in_=bass.AP(tensor=hs_ap.tensor, offset=hs_ap.offset,
                    ap=[[0, P], [1, D]]),
    )
    for ib in range(NIB):
        chx = gout.tile([P, D], F32, tag=f"ch{ib}", name=f"ch{ib}")
        nc.vector.tensor_tensor(out=chx, in0=c16[:, ib], in1=hb, op=ALU.mult)
        nc.sync.dma_start(out=g_d[ib * P:(ib + 1) * P, 3 * D:4 * D], in_=chx)

    ctx.close()


_NC_CACHE = {}


def _get_nc():
    if "nc" not in _NC_CACHE:
        _NC_CACHE["nc"] = build_kernel()
    return _NC_CACHE["nc"]


def _host_prep(x: np.ndarray, kern: np.ndarray):
    context = np.ascontiguousarray(x[0, 0]).astype(np.float32)   # (T, D)
    question = np.ascontiguousarray(x[1, 0]).astype(np.float32)  # (T, D)
    w = np.asarray(kern, dtype=np.float32)
    w2 = w[D:2 * D] * SC
    w3 = w[2 * D:3 * D] * SC

    q8 = question.astype(F8NP)
    r8 = (question - q8.astype(np.float32)).astype(F8NP)

    def punT(a8):
        # [T, D] fp8 -> [p, g, lo, j]: val = a8[j, g*256 + 2p + lo]
        v = a8.reshape(T, 2, P, 2)               # j, g, p, lo
        return np.ascontiguousarray(v.transpose(2, 1, 3, 0))

    def punW(a8):
        # [TL, D] fp8 -> [p, K=(ib,g), lo, f]: val = a8[ib*128+f, g*256+2p+lo]
        v = a8.reshape(NIB, P, 2, P, 2)          # ib, f, g, p, lo
        return np.ascontiguousarray(v.transpose(3, 0, 2, 4, 1)
                                    .reshape(P, 2 * NIB, 2, P))

    qnat = np.ascontiguousarray(
        q8.reshape(NJT, P, D).transpose(1, 0, 2))          # [p, jt, d]
    qt = punT(q8)
    rt = punT(r8)

    in_maps = []
    for core in range(NCORES):
        c = np.ascontiguousarray(context[core * TL:(core + 1) * TL])
        c16f = c.astype(np.float16)
        c16 = np.ascontiguousarray(
            c16f.reshape(NIB, P, D).transpose(1, 0, 2))    # [p, ib, d]
        wfull = (c16f.astype(np.float32) * w3[None, :] + w2[None, :])
        w8 = wfull.astype(F8NP)
        v8 = (wfull - w8.astype(np.float32)).astype(F8NP)
        in_maps.append({
            "qnat": qnat, "qt": qt, "rt": rt,
            "c16": c16, "w8": punW(w8), "v8": punW(v8), "c32": c,
        })
    return in_maps


def kernel(x: np.ndarray, kernel: np.ndarray) -> np.ndarray:
    nc = _get_nc()
    in_maps = _host_prep(x, kernel)
    res = run_bass_kernel_spmd(nc, in_maps, core_ids=list(range(NCORES)))
    g = np.concatenate([res.results[core]["g"] for core in range(NCORES)],
                       axis=0)
    return g.astype(np.float32)
